# revision 1
# baseline (speedup 1.0000x reference)
"""Trainium2 Bass kernel for the news-attention module.

Computes, per batch b:
    hist = [history_repr | pos_emb[positions]]            [H, 500]
    cand = [candidate_repr | pos_emb[1]]                  [N, 500]
    hc = cand @ Wc.T ; hh = hist @ Wh.T                   [*, 200]
    a[n,h] = w2 . relu(hc[n] + hh[h] + b1)
    alpha = softmax_h(mask ? a : -1e9)
    out1 = alpha @ hist ; out2 = cand

Structure:
  - position gather folded into matmuls: pos part of hh = onehot(pos) @ E
    with E = pos_emb @ Wh2.T; candidate pos part + b1 folded into a
    per-partition bias column c0 applied during PSUM evacuation.
  - all fp32 matmuls stream as float32r (full-rate fp32 mode for moving
    free dim >= 256); psum accumulation is fp32 either way.
  - hidden built in [A-chunk, (n, h)] layout with zero-stride broadcast
    APs on the DVE; hc stored with each element duplicated (pairs) so the
    innermost AP dim is packed, enabling the DVE 2-byte fast path in bf16
    mode; relu in place.
  - w2 contraction in column form: lhsT = hidden chunk (100 pairs),
    rhs = w2 column, accumulating into a [100, 25] psum per batch ->
    logits leave PSUM via one cheap 100-lane copy instead of 40
    single-lane row copies.
  - softmax over h batched for all batches in [25, parity, b, h] layout
    (n = 2c + parity); mask bias tile built in the same layout.
  - alpha @ hist as two parity matmuls per batch with 1/sum folded into
    the PSUM-evacuation scale.

Sharding: data-parallel over batch, 8 batches per core on 8 cores.
Params replicated. Full inputs in, full outputs out.
"""

import sys

for _p in ("/opt/trn_rl_repo",):
    if _p not in sys.path:
        sys.path.insert(0, _p)

import numpy as np

import concourse.bass as bass
import concourse.bacc as bacc
import concourse.tile as tile
from concourse import mybir
from concourse import bass_utils
from concourse.masks import make_identity

DT = mybir.dt.float32
FR = mybir.dt.float32r
BF = mybir.dt.bfloat16
I32 = mybir.dt.int32
AF = mybir.ActivationFunctionType
ALU = mybir.AluOpType
AX = mybir.AxisListType

NCORES = 8
B = 64
BC = B // NCORES  # 8 batches per core
H = 50
N = 50
D = 400
P = 100
A = 200
F = D + P       # 500
J = 52
NC2 = N // 2    # 25 pair-chunks of 100 pairs (2 candidates) per batch

# bf16 hidden pipeline (hc/hh/hidden/w2 in bf16; everything else fp32)
USE_BF16 = False
SKIP = set()  # timing ablations: {"hidden","matvec","gemm","transp","final"}
MATVEC_ROW = False  # row-form matvec (fewer, wider PE ops + ACT evac)
HDT = BF if USE_BF16 else DT


def _bc(v, pos, n):
    """Insert a zero-stride (broadcast) dim of length n at position pos."""
    ap = [list(x) for x in v.ap]
    ap.insert(pos, [0, n])
    return bass.AP(tensor=v.tensor, offset=v.offset, ap=ap)


def _ap(v, offset_delta, ap_list):
    return bass.AP(tensor=v.tensor, offset=v.offset + offset_delta, ap=ap_list)


def _r(ap):
    """Placeholder for fp32->float32r streaming (needs producer-side
    rounding on HW; disabled)."""
    return ap


def _body(nc, hist_in, cand_in, mask_in, pos_in, pos_emb, w1t, pos_embT,
          b1, w2, ur_out, cand_out, tc):
    import contextlib

    ctx = contextlib.ExitStack()
    with ctx:
        consts = ctx.enter_context(tc.tile_pool(name="consts", bufs=1))
        ps = ctx.enter_context(tc.tile_pool(name="ps", bufs=4, space="PSUM"))
        psm = ctx.enter_context(tc.tile_pool(name="psm", bufs=2, space="PSUM"))
        hidp = ctx.enter_context(tc.tile_pool(name="hid", bufs=3))
        smp = ctx.enter_context(tc.tile_pool(name="smp", bufs=1))
        amcp = ctx.enter_context(tc.tile_pool(name="amcp", bufs=2))
        eTp = ctx.enter_context(tc.tile_pool(name="eTp", bufs=3))

        # ---------------- constants ----------------
        ident = consts.tile([128, 128], DT)
        make_identity(nc, ident)

        # W1T[f, a] in 10 f-chunks of 100 (host provides W1 transposed)
        w1T = consts.tile([100, 10, A], DT)
        nc.sync.dma_start(out=w1T,
                          in_=w1t.ap().rearrange("(k p) a -> p k a", p=100))

        pos_emb_s = consts.tile([J, P], DT)
        nc.sync.dma_start(out=pos_emb_s, in_=pos_emb.ap())
        posT = consts.tile([P, J], DT)
        nc.sync.dma_start(out=posT, in_=pos_embT.ap())

        # E[j, a] = pos_emb @ Wh2.T  (Wh2 = W1[:, 900:1000])
        E_s = consts.tile([J, A], DT)
        psE = ps.tile([J, A], DT, tag="ps")
        nc.tensor.matmul(psE, lhsT=_r(posT[:, :]), rhs=_r(w1T[:, 9, :]),
                         start=True, stop=True)
        nc.vector.tensor_copy(out=E_s, in_=psE)

        b1row = consts.tile([1, A], DT)
        nc.sync.dma_start(out=b1row, in_=b1.ap())
        one11 = consts.tile([1, 1], DT)
        nc.vector.memset(one11, 1.0)

        # c0[a] = Wc2 @ pos_emb[1] + b1 as two per-partition bias columns
        c0col = consts.tile([100, 2], DT)
        for ac in range(2):
            asl = slice(ac * 100, (ac + 1) * 100)
            psc = ps.tile([100, 1], DT, tag="ps")
            nc.tensor.matmul(psc, lhsT=_r(w1T[:, 4, asl]), rhs=_r(posT[:, 1:2]),
                             start=True, stop=False)
            nc.tensor.matmul(psc, lhsT=_r(b1row[:, asl]), rhs=_r(one11[:, :]),
                             start=False, stop=True)
            nc.scalar.copy(out=c0col[:, ac:ac + 1], in_=psc)

        w2col = consts.tile([100, 2], HDT)
        nc.gpsimd.dma_start(out=w2col,
                            in_=w2.ap().rearrange("(c p) -> p c", p=100))

        # mask bias (mask-1)*1e9 in [c, q, b, h] layout (broadcast over c, q)
        mb25 = consts.tile([NC2, 2, BC, H], DT)
        nc.gpsimd.dma_start(out=mb25, in_=_bc(_bc(mask_in.ap(), 0, 2), 0, NC2))
        nc.scalar.activation(out=mb25, in_=mb25, func=AF.Copy,
                             bias=-1e9, scale=1e9)

        # one-hot of positions, transposed: onehot[j, b*H+h] = (pos[b,h]==j)
        pos52 = consts.tile([J, BC * H], I32)
        nc.gpsimd.dma_start(out=pos52, in_=_bc(pos_in.ap(), 0, J))
        iot = consts.tile([J, BC * H], I32)
        nc.gpsimd.iota(iot, pattern=[[0, BC * H]], base=0, channel_multiplier=1)
        onehot_s = consts.tile([J, BC * H], DT)
        nc.vector.tensor_tensor(out=onehot_s, in0=iot, in1=pos52, op=ALU.is_equal)

        # ---------------- data load + transpose ----------------
        cand_all = consts.tile([100, 4, D], DT)   # [2x50 rows, batch-pair, feat]
        hist_all = consts.tile([100, 4, D], DT)
        for hf in range(2):
            sl = slice(hf * 50, (hf + 1) * 50)
            src_c = _ap(cand_in.ap(), hf * N * D,
                        [[D, 50], [2 * N * D, 4], [1, D]])
            src_h = _ap(hist_in.ap(), hf * H * D,
                        [[D, 50], [2 * H * D, 4], [1, D]])
            nc.sync.dma_start(out=cand_all[sl, :, :], in_=src_c)
            nc.sync.dma_start(out=hist_all[sl, :, :], in_=src_h)

        candT = consts.tile([100, 4, BC * N], DT)  # [feat-chunk, k, (b,n)]
        histT = consts.tile([100, 4, BC * H], DT)
        for g in range(4 if "transp" not in SKIP else 0):
            ptc = ps.tile([100, 4, 100], DT, tag="ps")
            pth = ps.tile([100, 4, 100], DT, tag="ps")
            for k in range(4):
                nc.tensor.transpose(
                    ptc[:, k, :],
                    _r(cand_all[:, g, k * 100:(k + 1) * 100]),
                    _r(ident[:100, :100]))
                nc.tensor.transpose(
                    pth[:, k, :],
                    _r(hist_all[:, g, k * 100:(k + 1) * 100]),
                    _r(ident[:100, :100]))
            nc.scalar.copy(out=candT[:, :, g * 100:(g + 1) * 100], in_=ptc)
            nc.scalar.copy(out=histT[:, :, g * 100:(g + 1) * 100], in_=pth)

        # hist with position columns, natural layout, all batches
        histf_all = consts.tile([H, BC, F], DT)
        src_hf = _ap(hist_in.ap(), 0, [[D, H], [H * D, BC], [1, D]])
        nc.sync.dma_start(out=histf_all[:, :, 0:D], in_=src_hf)
        for b in range(BC):
            ppg = ps.tile([H, P], DT, tag="ps")
            nc.tensor.matmul(ppg, lhsT=_r(onehot_s[:, b * H:(b + 1) * H]),
                             rhs=_r(pos_emb_s[:, :]), start=True, stop=True)
            nc.scalar.copy(out=histf_all[:, b, D:F], in_=ppg)

        # candidate passthrough: two strided DMAs + pos_emb[1] broadcast
        for hf in range(2):
            dst = _ap(cand_out.ap(), hf * N * F,
                      [[F, 50], [2 * N * F, 4], [1, D]])
            nc.sync.dma_start(out=dst, in_=cand_all[hf * 50:(hf + 1) * 50, :, :])
        nc.gpsimd.dma_start(
            out=cand_out.ap()[:, :, D:F],
            in_=_bc(_bc(pos_emb.ap()[1:2, :], 0, N), 0, BC))

        # ------- GEMMs: hcT2[a, (b,n), dup2] (duplicated), hhT[a, (b,h)] ----
        hcT2 = consts.tile([100, 2, BC * N, 2], HDT)
        hhT = consts.tile([100, 2, BC * H], HDT)
        for ac in range(2 if "gemm" not in SKIP else 0):
            asl = slice(ac * 100, (ac + 1) * 100)
            pg = ps.tile([100, BC * N], DT, tag="ps")
            for k in range(4):
                nc.tensor.matmul(pg, lhsT=_r(w1T[:, k, asl]),
                                 rhs=_r(candT[:, k, :]),
                                 start=(k == 0), stop=(k == 3))
            # evacuate + add c0 bias, duplicating each element (dup2 dim)
            nc.scalar.activation(out=hcT2[:, ac, :, :], in_=_bc(pg[:, :], 2, 2),
                                 func=AF.Identity, bias=c0col[:, ac:ac + 1],
                                 scale=1.0)

            ph = ps.tile([100, BC * H], DT, tag="ps")
            for k in range(4):
                nc.tensor.matmul(ph, lhsT=_r(w1T[:, 5 + k, asl]),
                                 rhs=_r(histT[:, k, :]),
                                 start=(k == 0), stop=False)
            nc.tensor.matmul(ph, lhsT=_r(E_s[:, asl]), rhs=_r(onehot_s[:, :]),
                             start=False, stop=True)
            nc.scalar.copy(out=hhT[:, ac, :], in_=ph)

        # ---------------- hidden + relu + w2 column matvec ----------------
        scratch = None
        if MATVEC_ROW:
            scratch = nc.dram_tensor(
                f"scratch{nc.next_id()}", [BC, N, H], DT)
        amr = smp.tile([NC2, 2, BC, H], DT, tag="amr")
        for b in range(BC):
            hids = []
            for ac in range(2):
                # hc broadcast over h via duplicated pairs: free = (n, hq, hr)
                v = hcT2[:, ac, b * N:(b + 1) * N, :]   # [[p],[2,50],[1,2]]
                hcb = _bc(v, 2, H // 2)                 # [100, 50, 25, 2]
                w = hhT[:, ac, b * H:(b + 1) * H]       # [[p],[1,50]]
                hhb = _ap(w, 0, [list(w.ap[0]), [0, N], [2, H // 2], [1, 2]])
                hid = hidp.tile([100, N, H // 2, 2], HDT, tag=f"hid{ac}")
                if "hidden" not in SKIP:
                    nc.vector.tensor_add(out=hid, in0=hcb, in1=hhb)
                    nc.vector.tensor_scalar_max(out=hid, in0=hid, scalar1=0.0)
                hids.append(hid)
            if MATVEC_ROW:
                arow = amcp.tile([1, 5, 500], DT, tag="arow")
                for gn in range(5 if "matvec" not in SKIP else 0):
                    psa = psm.tile([1, 500], DT, tag="psa")
                    for ac in range(2):
                        hv = hids[ac]
                        rhs = hv[:, gn * 10:(gn + 1) * 10, :, :]
                        nc.tensor.matmul(psa, lhsT=_r(w2col[:, ac:ac + 1]),
                                         rhs=_r(rhs),
                                         start=(ac == 0), stop=(ac == 1))
                    nc.scalar.copy(out=arow[0:1, gn, :], in_=psa)
                if "matvec" in SKIP:
                    continue
                nc.sync.dma_start(out=scratch.ap()[b], in_=arow)
                continue
            amc = psm.tile([100, NC2], DT, tag="amc")
            for c in range(NC2 if "matvec" not in SKIP else 0):
                for ac in range(2):
                    hv = hids[ac]
                    pst = [list(x) for x in hv.ap][0]
                    lhs = _ap(hv, c * 100, [pst, [1, 100]])
                    nc.tensor.matmul(amc[:, c:c + 1], lhsT=_r(lhs),
                                     rhs=_r(w2col[:, ac:ac + 1]),
                                     start=(ac == 0), stop=(ac == 1))
            amcs = amcp.tile([100, NC2], DT, tag="amcs")
            if "matvec" in SKIP:
                continue
            nc.vector.tensor_copy(out=amcs, in_=amc)
            amT = psm.tile([NC2, 2 * H], DT, tag="amT")
            nc.tensor.transpose(amT[:, :], _r(amcs[:, :]),
                                _r(ident[:100, :100]))
            nc.vector.tensor_copy(out=amr[:, :, b, :], in_=amT)
        if MATVEC_ROW and "matvec" not in SKIP:
            for q in range(2):
                src_q = _ap(scratch.ap(), q * H,
                            [[2 * H, NC2], [N * H, BC], [1, H]])
                nc.sync.dma_start(out=amr[:, q, :, :], in_=src_q)

        # ---------------- batched mask + softmax over h ----------------
        amm = smp.tile([NC2, 2, BC, H], DT, tag="amm")
        nc.vector.tensor_add(out=amm, in0=amr, in1=mb25)
        nm = smp.tile([NC2, 2, BC], DT, tag="nm")
        nc.vector.tensor_reduce(out=nm, in_=amm, axis=AX.X, op=ALU.max,
                                negate=True)
        am2 = smp.tile([NC2, 2, BC, H], DT, tag="am2")
        nc.vector.tensor_add(out=am2, in0=amm, in1=_bc(nm[:, :, :], 3, H))
        ex = smp.tile([NC2, 2, BC, H], DT, tag="ex")
        nc.scalar.activation(out=ex, in_=am2, func=AF.Exp)
        ssum = smp.tile([NC2, 2, BC], DT, tag="ssum")
        nc.vector.tensor_reduce(out=ssum, in_=ex, axis=AX.X, op=ALU.add)
        rs = smp.tile([NC2, 2, BC], DT, tag="rs")
        nc.vector.reciprocal(rs, ssum)

        # ---------------- attention-weighted history ----------------
        urs_all = consts.tile([NC2, 2, BC, F], DT)
        for b in range(BC if "final" not in SKIP else 0):
            for q in range(2):
                peT = ps.tile([H, NC2], DT, tag="ps")
                nc.tensor.transpose(peT[:, :], _r(ex[:, q, b, :]),
                                    _r(ident[:NC2, :NC2]))
                eT = eTp.tile([H, NC2], DT, tag="eT")
                nc.vector.tensor_copy(out=eT, in_=peT)

                pur = ps.tile([NC2, F], DT, tag="ps")
                nc.tensor.matmul(pur, lhsT=_r(eT[:, :]),
                                 rhs=_r(histf_all[:, b, :]),
                                 start=True, stop=True)
                nc.scalar.activation(out=urs_all[:, q, b, :], in_=pur,
                                     func=AF.Copy, scale=rs[:, q, b:b + 1])
        dst_ur = _ap(ur_out.ap(), 0,
                     [[2 * F, NC2], [F, 2], [N * F, BC], [1, F]])
        nc.sync.dma_start(out=dst_ur, in_=urs_all)


def build(debug=False, reps=1):
    nc = bacc.Bacc("TRN2", target_bir_lowering=False, debug=debug)
    hist_in = nc.dram_tensor("hist_in", [BC, H, D], DT, kind="ExternalInput")
    cand_in = nc.dram_tensor("cand_in", [BC, N, D], DT, kind="ExternalInput")
    mask_in = nc.dram_tensor("mask_in", [BC, H], DT, kind="ExternalInput")
    pos_in = nc.dram_tensor("pos_in", [BC, H], I32, kind="ExternalInput")
    pos_emb = nc.dram_tensor("pos_emb", [J, P], DT, kind="ExternalInput")
    w1t = nc.dram_tensor("w1t", [2 * F, A], DT, kind="ExternalInput")
    pos_embT = nc.dram_tensor("pos_embT", [P, J], DT, kind="ExternalInput")
    b1 = nc.dram_tensor("b1", [A], DT, kind="ExternalInput")
    w2 = nc.dram_tensor("w2", [A], DT, kind="ExternalInput")
    ur_out = nc.dram_tensor("ur_out", [BC, N, F], DT, kind="ExternalOutput")
    cand_out = nc.dram_tensor("cand_out", [BC, N, F], DT, kind="ExternalOutput")

    with tile.TileContext(nc) as tc:
        for _ in range(reps):
            _body(nc, hist_in, cand_in, mask_in, pos_in, pos_emb, w1t,
                  pos_embT, b1, w2, ur_out, cand_out, tc)
    nc.compile()
    return nc


_NC = None


def _get_nc():
    global _NC
    if _NC is None:
        _NC = build(debug=False)
    return _NC


def make_in_maps(history_repr, candidate_repr, user_history_mask,
                 user_history_position, pos_emb, W1, b1, w2):
    hist = np.ascontiguousarray(np.asarray(history_repr, np.float32))
    cand = np.ascontiguousarray(np.asarray(candidate_repr, np.float32))
    mask = np.asarray(user_history_mask).astype(np.float32)
    pos = np.asarray(user_history_position).astype(np.int32)
    pe = np.ascontiguousarray(np.asarray(pos_emb, np.float32))
    w1t = np.ascontiguousarray(np.asarray(W1, np.float32).T)
    peT = np.ascontiguousarray(pe.T)
    b1_ = np.ascontiguousarray(np.asarray(b1, np.float32))
    w2_ = np.ascontiguousarray(np.asarray(w2, np.float32))
    in_maps = []
    for c in range(NCORES):
        sl = slice(c * BC, (c + 1) * BC)
        in_maps.append({
            "hist_in": hist[sl], "cand_in": cand[sl],
            "mask_in": mask[sl], "pos_in": pos[sl],
            "pos_emb": pe, "w1t": w1t, "pos_embT": peT,
            "b1": b1_, "w2": w2_,
        })
    return in_maps


def kernel(history_repr, candidate_repr, user_history_mask,
           user_history_position, pos_emb, W1, b1, w2, b2=None, **_ignored):
    # b2 shifts every logit equally -> cancels in softmax; unused.
    nc = _get_nc()
    in_maps = make_in_maps(history_repr, candidate_repr, user_history_mask,
                           user_history_position, pos_emb, W1, b1, w2)
    res = bass_utils.run_bass_kernel_spmd(nc, in_maps, list(range(NCORES)))
    ur = np.concatenate([res.results[c]["ur_out"] for c in range(NCORES)], 0)
    cand = np.concatenate([res.results[c]["cand_out"] for c in range(NCORES)], 0)
    return ur, cand



# revision 2
# speedup vs baseline: 13.3813x; 13.3813x over previous
"""Trainium2 Bass kernel for the news-attention module (v2).

Math restructuring vs baseline:
  relu(hc + hh) = max(hc, -hh) + hh          (exact identity)
  logits a[n,h] = w2 . max(hc[n], -hh[h]) + sh[h],  sh = w2 . hh
  softmax(a + maskbias) = exp(w2.max-part) * m[h] / sum,
      m[h] = mask[h] * exp(sh[h])            (multiplicative fold)
  ur[n] = (1/ssum) sum_h exnorm[n,h] * (m[h] * histf[h])   -> m folds
      into the per-partition ACT scale of the histf evacuation, and
      ssum comes from a ones-column appended to histf.

So the hidden tensor needs ONE bf16 DVE tensor_tensor(max) pass (2x mode
via dup2-packed hc), the w2-matvec runs as row-form streaming matmuls
into a slot-packed [40, 500] psum (slot = ftile*8 + batch, via a shifted
zero-window lhsT), and a single ACT Exp instruction evacuates all
logits.  No relu pass, no separate bias add, no masked softmax pass.

Sharding: data-parallel over batch, 8 batches per core on 8 cores.
"""

import sys

for _p in ("/opt/trn_rl_repo",):
    if _p not in sys.path:
        sys.path.insert(0, _p)

import numpy as np
import ml_dtypes

import concourse.bass as bass
import concourse.bacc as bacc
import concourse.tile as tile
from concourse import mybir
from concourse import bass_utils
from concourse.masks import make_identity

DT = mybir.dt.float32
BF = mybir.dt.bfloat16
I32 = mybir.dt.int32
AF = mybir.ActivationFunctionType
ALU = mybir.AluOpType
AX = mybir.AxisListType

NCORES = 8
B = 64
BC = B // NCORES  # 8 batches per core
H = 50
N = 50
D = 400
P = 100
A = 200
F = D + P       # 500
J = 52
A0 = 128        # a-chunk split: [0:128), [128:200)
A1 = A - A0     # 72
NT = 7          # matvec f-tiles per batch (8 n x 64 h-padded psum cols)
NSLOT = NT * BC  # 56 psum slots


def _bc_(v, pos, n):
    """Insert a zero-stride (broadcast) dim of length n at position pos."""
    ap = [list(x) for x in v.ap]
    ap.insert(pos, [0, n])
    return bass.AP(tensor=v.tensor, offset=v.offset, ap=ap)


def _ap(v, offset_delta, ap_list):
    return bass.AP(tensor=v.tensor, offset=v.offset + offset_delta, ap=ap_list)


def _body(nc, hist_in, cand_in, maskT_in, pos_in, pos_emb16, pos_emb32,
          posT16, w1t, b1_16, w2_16, ur_out, cand_out, tc):
    import contextlib

    ctx = contextlib.ExitStack()
    with ctx:
        consts = ctx.enter_context(tc.tile_pool(name="consts", bufs=1))
        ps = ctx.enter_context(tc.tile_pool(name="ps", bufs=3, space="PSUM"))
        psmv = ctx.enter_context(tc.tile_pool(name="psmv", bufs=2, space="PSUM"))
        psat = ctx.enter_context(tc.tile_pool(name="psat", bufs=2, space="PSUM"))
        hidp = ctx.enter_context(tc.tile_pool(name="hid", bufs=3))

        # ---------------- constants ----------------
        ident16 = consts.tile([128, 128], BF)
        make_identity(nc, ident16)
        ident32 = consts.tile([128, 128], DT)
        make_identity(nc, ident32)

        # W1T[f, a] in 10 f-chunks of 100 (host provides W1 transposed, bf16)
        w1T = consts.tile([100, 10, A], BF)
        nc.sync.dma_start(out=w1T,
                          in_=w1t.ap().rearrange("(k p) a -> p k a", p=100))

        pos_emb_s = consts.tile([J, P], BF)
        nc.sync.dma_start(out=pos_emb_s, in_=pos_emb16.ap())
        posT = consts.tile([P, J], BF)
        nc.sync.dma_start(out=posT, in_=posT16.ap())
        maskT_s = consts.tile([H, BC], DT)
        nc.sync.dma_start(out=maskT_s, in_=maskT_in.ap())
        b1row = consts.tile([1, A], BF)
        nc.sync.dma_start(out=b1row, in_=_ap(b1_16.ap(), 0, [[0, 1], [1, A]]))
        one11 = consts.tile([1, 1], BF)
        nc.vector.memset(one11, 1.0)

        # w2 chunks as columns + shifted zero-window tensors for slot matvec
        w2colA = consts.tile([A0, 1], BF)
        nc.sync.dma_start(out=w2colA, in_=_ap(w2_16.ap(), 0, [[1, A0], [1, 1]]))
        w2colB = consts.tile([A1, 1], BF)
        nc.sync.dma_start(out=w2colB, in_=_ap(w2_16.ap(), A0, [[1, A1], [1, 1]]))
        ZA = consts.tile([A0, 2 * NSLOT - 1], BF)
        ZB = consts.tile([A1, 2 * NSLOT - 1], BF)
        nc.vector.memset(ZA, 0.0)
        nc.vector.memset(ZB, 0.0)
        nc.vector.tensor_copy(out=ZA[:, NSLOT - 1:NSLOT], in_=w2colA)
        nc.vector.tensor_copy(out=ZB[:, NSLOT - 1:NSLOT], in_=w2colB)

        # E[j, a] = pos_emb @ Wh2.T  (Wh2 = W1[:, 900:1000] rows of w1t)
        E_s = consts.tile([J, A], BF)
        psE = ps.tile([J, A], DT, tag="ps")
        nc.tensor.matmul(psE, lhsT=posT[:, :], rhs=w1T[:, 9, :],
                         start=True, stop=True)
        nc.scalar.copy(out=E_s, in_=psE)

        # c0[a] = Wc2 @ pos_emb[1] + b1 as per-partition bias columns
        c0colA = consts.tile([A0, 1], DT)
        c0colB = consts.tile([A1, 1], DT)
        for c0c, asl in ((c0colA, slice(0, A0)), (c0colB, slice(A0, A))):
            psc = ps.tile([c0c.shape[0], 1], DT, tag="ps")
            nc.tensor.matmul(psc, lhsT=w1T[:, 4, asl], rhs=posT[:, 1:2],
                             start=True, stop=False)
            nc.tensor.matmul(psc, lhsT=b1row[:, asl], rhs=one11[:, :],
                             start=False, stop=True)
            nc.scalar.copy(out=c0c, in_=psc)

        # one-hot of positions, transposed: onehot[j, b*H+h] = (pos[b,h]==j)
        pos52 = consts.tile([J, BC * H], I32)
        nc.gpsimd.dma_start(out=pos52, in_=_bc_(pos_in.ap(), 0, J))
        iot = consts.tile([J, BC * H], I32)
        nc.gpsimd.iota(iot, pattern=[[0, BC * H]], base=0, channel_multiplier=1)
        onehot = consts.tile([J, BC * H], BF)
        nc.vector.tensor_tensor(out=onehot, in0=iot, in1=pos52, op=ALU.is_equal)

        # ---------------- data load + transpose (fp32 in, bf16 out) -------
        cand_all = consts.tile([100, 4, D], DT)   # [2x50 rows, batch-pair, feat]
        hist_all = consts.tile([100, 4, D], DT)
        for hf in range(2):
            sl = slice(hf * 50, (hf + 1) * 50)
            src_c = _ap(cand_in.ap(), hf * N * D,
                        [[D, 50], [2 * N * D, 4], [1, D]])
            src_h = _ap(hist_in.ap(), hf * H * D,
                        [[D, 50], [2 * H * D, 4], [1, D]])
            nc.sync.dma_start(out=cand_all[sl, :, :], in_=src_c)
            nc.sync.dma_start(out=hist_all[sl, :, :], in_=src_h)

        candT = consts.tile([100, 4, BC * N], BF)  # [feat-chunk, k, (b,n)]
        histT = consts.tile([100, 4, BC * H], BF)
        for g in range(4):
            ptc = ps.tile([100, 4, 100], DT, tag="ps")
            pth = ps.tile([100, 4, 100], DT, tag="ps")
            for k in range(4):
                nc.tensor.transpose(
                    ptc[:, k, :], cand_all[:, g, k * 100:(k + 1) * 100],
                    ident32[:100, :100])
                nc.tensor.transpose(
                    pth[:, k, :], hist_all[:, g, k * 100:(k + 1) * 100],
                    ident32[:100, :100])
            nc.scalar.copy(out=candT[:, :, g * 100:(g + 1) * 100], in_=ptc)
            nc.scalar.copy(out=histT[:, :, g * 100:(g + 1) * 100], in_=pth)

        # candidate passthrough: two strided DMAs + pos_emb[1] broadcast
        for hf in range(2):
            dst = _ap(cand_out.ap(), hf * N * F,
                      [[F, 50], [2 * N * F, 4], [1, D]])
            nc.sync.dma_start(out=dst, in_=cand_all[hf * 50:(hf + 1) * 50, :, :])
        nc.gpsimd.dma_start(
            out=cand_out.ap()[:, :, D:F],
            in_=_bc_(_bc_(pos_emb32.ap()[1:2, :], 0, N), 0, BC))

        # ---------------- GEMMs -> hcT2 (dup2, +c0), neghh ----------------
        hcT2A = consts.tile([A0, BC * N, 2], BF)
        hcT2B = consts.tile([A1, BC * N, 2], BF)
        neghhA = consts.tile([A0, BC * H], BF)
        neghhB = consts.tile([A1, BC * H], BF)
        for asl, hcT2, neghh in ((slice(0, A0), hcT2A, neghhA),
                                 (slice(A0, A), hcT2B, neghhB)):
            m = asl.stop - asl.start
            pg = ps.tile([m, BC * N], DT, tag="ps")
            for k in range(4):
                nc.tensor.matmul(pg, lhsT=w1T[:, k, asl], rhs=candT[:, k, :],
                                 start=(k == 0), stop=(k == 3))
            nc.scalar.activation(out=hcT2, in_=_bc_(pg[:, :], 2, 2),
                                 func=AF.Identity,
                                 bias=c0colA if asl.start == 0 else c0colB,
                                 scale=1.0)
            ph = ps.tile([m, BC * H], DT, tag="ps")
            for k in range(4):
                nc.tensor.matmul(ph, lhsT=w1T[:, 5 + k, asl],
                                 rhs=histT[:, k, :],
                                 start=(k == 0), stop=False)
            nc.tensor.matmul(ph, lhsT=E_s[:, asl], rhs=onehot[:, :],
                             start=False, stop=True)
            nc.scalar.activation(out=neghh, in_=ph, func=AF.Copy, scale=-1.0)

        # shT[h, b] = w2 . neghh  (column-form tiny MMs) -> m = mask*exp(-shT)
        psh = ps.tile([H, BC], DT, tag="ps")
        for b in range(BC):
            hsl = slice(b * H, (b + 1) * H)
            nc.tensor.matmul(psh[:, b:b + 1], lhsT=neghhA[:, hsl], rhs=w2colA,
                             start=(True), stop=False)
            nc.tensor.matmul(psh[:, b:b + 1], lhsT=neghhB[:, hsl], rhs=w2colB,
                             start=False, stop=True)
        shE = consts.tile([H, BC], DT)
        nc.scalar.activation(out=shE, in_=psh, func=AF.Exp, scale=-1.0)
        mw = consts.tile([H, BC], DT)
        nc.vector.tensor_tensor(out=mw, in0=shE, in1=maskT_s, op=ALU.mult)

        # ---------------- histf (V matrix) with m-scale + ones col --------
        staging = consts.tile([H, BC, F + 1], DT)
        src_hf = _ap(hist_in.ap(), 0, [[D, H], [H * D, BC], [1, D]])
        nc.sync.dma_start(out=staging[:, :, 0:D], in_=src_hf)
        for b in range(BC):
            ppg = ps.tile([H, P], DT, tag="ps")
            nc.tensor.matmul(ppg, lhsT=onehot[:, b * H:(b + 1) * H],
                             rhs=pos_emb_s[:, :], start=True, stop=True)
            nc.vector.tensor_copy(out=staging[:, b, D:F], in_=ppg)
        nc.vector.memset(staging[:, :, F:F + 1], 1.0)
        histf16 = consts.tile([H, BC, F + 1], BF)
        for b in range(BC):
            nc.scalar.activation(out=histf16[:, b, :], in_=staging[:, b, :],
                                 func=AF.Copy, scale=mw[:, b:b + 1])

        # ---------------- hidden max-pass + slot-packed matvec ------------
        # psum [56, 8, 64]: slot s = t*8+b holds logits of batch b for
        # n in [8t, 8t+8) laid out (n-local, h) with h padded to 64.
        pmv = psmv.tile([NSLOT, 8, 64], DT, tag="pmv")
        first = True
        for b in range(BC):
            nsl = slice(b * N, (b + 1) * N)
            hids = []
            for hcT2, neghh, aw in ((hcT2A, neghhA, A0), (hcT2B, neghhB, A1)):
                hid = hidp.tile([aw, N * H], BF, tag=f"hid{aw}")
                v = hcT2[:, nsl, :]
                hcb = _bc_(v, 2, H // 2)                # [aw, 50, 25, 2]
                w = neghh[:, b * H:(b + 1) * H]
                hhb = _ap(w, 0, [list(w.ap[0]), [0, N], [2, H // 2], [1, 2]])
                hidv = _ap(hid, 0, [list(hid.ap[0]), [H, N], [2, H // 2],
                                    [1, 2]])
                nc.vector.tensor_tensor(out=hidv, in0=hcb, in1=hhb, op=ALU.max)
                hids.append(hid)
            for t in range(NT):
                s = t * BC + b
                nn = 8 if t < 6 else 2       # n-cols in this tile
                rsl = slice(t * 8 * H, (t * 8 + nn) * H)
                dst = _ap(pmv, 0, [list(pmv.ap[0]), [64, nn], [1, H]])
                for Z, hv in ((ZA, hids[0]), (ZB, hids[1])):
                    last = (b == BC - 1 and t == NT - 1 and Z is ZB)
                    nc.tensor.matmul(
                        dst, lhsT=Z[:, NSLOT - 1 - s:2 * NSLOT - 1 - s],
                        rhs=hv[:, rsl], start=first, stop=last)
                    first = False

        # one ACT instruction evacuates+exps ALL logits
        ex = consts.tile([NSLOT, 8, 64], BF)
        nc.scalar.activation(out=ex[:, :, 0:H], in_=pmv[:, :, 0:H],
                             func=AF.Exp)

        # ---------------- transpose ex -> alphaT[h, b, n] ----------------
        # window w = 128 cols = 2 n-locals x 64 h -> out partitions
        # (n-parity, h64): h-runs at partition 0 and 64.
        ptr = ps.tile([128, 4, NSLOT], BF, tag="ps")
        for w in range(4):
            nc.tensor.transpose(
                ptr[:, w, :],
                _ap(ex, w * 128, [list(ex.ap[0]), [1, 128]]),
                ident16[:NSLOT, :NSLOT])
        alphaT = consts.tile([H, BC, 64], BF)
        for w in range(4):
            for par in range(2):
                src = ptr[par * 64:par * 64 + 50, :, :]
                nc.vector.tensor_copy(
                    out=_ap(alphaT, 2 * w + par,
                            [list(alphaT.ap[0]), [8, NT], [64, BC]]),
                    in_=_ap(src, w * NSLOT, [list(src.ap[0]), [BC, NT],
                                             [1, BC]]))

        # ---------------- attention: ur = alpha @ histf / ssum ------------
        urs = consts.tile([N, BC, F], DT)
        rs_all = consts.tile([N, BC], DT)
        for b in range(BC):
            pur = psat.tile([N, F + 1], DT, tag="pur")
            nc.tensor.matmul(pur, lhsT=alphaT[:, b, 0:N],
                             rhs=histf16[:, b, :], start=True, stop=True)
            nc.vector.reciprocal(rs_all[:, b:b + 1], pur[:, F:F + 1])
            nc.scalar.activation(out=urs[:, b, :], in_=pur[:, 0:F],
                                 func=AF.Copy, scale=rs_all[:, b:b + 1])
        dst_ur = _ap(ur_out.ap(), 0, [[F, N], [N * F, BC], [1, F]])
        nc.sync.dma_start(out=dst_ur, in_=urs)


def build(debug=False, reps=1):
    nc = bacc.Bacc("TRN2", target_bir_lowering=False, debug=debug)
    hist_in = nc.dram_tensor("hist_in", [BC, H, D], DT, kind="ExternalInput")
    cand_in = nc.dram_tensor("cand_in", [BC, N, D], DT, kind="ExternalInput")
    maskT_in = nc.dram_tensor("maskT_in", [H, BC], DT, kind="ExternalInput")
    pos_in = nc.dram_tensor("pos_in", [BC, H], I32, kind="ExternalInput")
    pos_emb16 = nc.dram_tensor("pos_emb16", [J, P], BF, kind="ExternalInput")
    pos_emb32 = nc.dram_tensor("pos_emb32", [J, P], DT, kind="ExternalInput")
    posT16 = nc.dram_tensor("posT16", [P, J], BF, kind="ExternalInput")
    w1t = nc.dram_tensor("w1t", [2 * F, A], BF, kind="ExternalInput")
    b1_16 = nc.dram_tensor("b1_16", [A], BF, kind="ExternalInput")
    w2_16 = nc.dram_tensor("w2_16", [A], BF, kind="ExternalInput")
    ur_out = nc.dram_tensor("ur_out", [BC, N, F], DT, kind="ExternalOutput")
    cand_out = nc.dram_tensor("cand_out", [BC, N, F], DT, kind="ExternalOutput")

    with tile.TileContext(nc) as tc:
        for _ in range(reps):
            _body(nc, hist_in, cand_in, maskT_in, pos_in, pos_emb16,
                  pos_emb32, posT16, w1t, b1_16, w2_16, ur_out, cand_out, tc)
    nc.compile()
    return nc


_NC = None


def _get_nc():
    global _NC
    if _NC is None:
        _NC = build(debug=False)
    return _NC


def _bf(x):
    return np.ascontiguousarray(np.asarray(x, np.float32).astype(
        ml_dtypes.bfloat16))


def make_in_maps(history_repr, candidate_repr, user_history_mask,
                 user_history_position, pos_emb, W1, b1, w2):
    hist = np.ascontiguousarray(np.asarray(history_repr, np.float32))
    cand = np.ascontiguousarray(np.asarray(candidate_repr, np.float32))
    mask = np.asarray(user_history_mask).astype(np.float32)
    pos = np.asarray(user_history_position).astype(np.int32)
    pe32 = np.ascontiguousarray(np.asarray(pos_emb, np.float32))
    pe16 = _bf(pe32)
    peT16 = _bf(pe32.T)
    w1t16 = _bf(np.asarray(W1, np.float32).T)
    b1_16 = _bf(b1)
    w2_16 = _bf(w2)
    in_maps = []
    for c in range(NCORES):
        sl = slice(c * BC, (c + 1) * BC)
        in_maps.append({
            "hist_in": hist[sl], "cand_in": cand[sl],
            "maskT_in": np.ascontiguousarray(mask[sl].T),
            "pos_in": pos[sl],
            "pos_emb16": pe16, "pos_emb32": pe32, "posT16": peT16,
            "w1t": w1t16, "b1_16": b1_16, "w2_16": w2_16,
        })
    return in_maps


def kernel(history_repr, candidate_repr, user_history_mask,
           user_history_position, pos_emb, W1, b1, w2, b2=None, **_ignored):
    # b2 shifts every logit equally -> cancels in softmax; unused.
    nc = _get_nc()
    in_maps = make_in_maps(history_repr, candidate_repr, user_history_mask,
                           user_history_position, pos_emb, W1, b1, w2)
    res = bass_utils.run_bass_kernel_spmd(nc, in_maps, list(range(NCORES)))
    ur = np.concatenate([res.results[c]["ur_out"] for c in range(NCORES)], 0)
    cand = np.concatenate([res.results[c]["cand_out"] for c in range(NCORES)], 0)
    return ur, cand
